# revision 6
# baseline (speedup 1.0000x reference)
"""DBF (binary-weight) MLP kernel for 8 TRN2 NeuronCores.

Computation (see reference):
    h   = (x * s0) @ W1.T          W1 = 2*w1_bits - 1  (+-1)
    h   = h * s2
    out = h @ W3.T * s4 + bias     W3 = 2*w3_bits - 1  (+-1)

Strategy:
  - Data-parallel: shard the 4*2048 = 8192 tokens across 8 cores (1024 each).
    Weights/scalings replicated. No collectives.
  - Activations kept feature-major on chip ([feature, token]); both GEMMs then
    chain naturally on the tensor engine (contraction dim on partitions) with
    no transposes between layers.
  - +-1 weights are exact in bf16, so both GEMMs run in bf16 (moving operand
    = activations rounded to bf16; fp32 PSUM accumulation).
  - Weights are packed on the host into per-m-tile SBUF images so every DMA
    is a single fully contiguous 1 MiB transfer.
"""

import numpy as np
import ml_dtypes

B, S, IN, MID, OUT = 4, 2048, 4096, 4096, 4096
NCORES = 8
NTOK = B * S            # 8192 tokens
NPC = NTOK // NCORES    # 1024 tokens per core
P = 128
KT, MT, OT = IN // P, MID // P, OUT // P   # 32 tiles each
FD = 512                # matmul moving free dim (one PSUM bank of fp32)

_cache = {}


def _pack_weight(w_bits: np.ndarray) -> np.ndarray:
    """[R, C] 0/1 int32 -> per-row-tile SBUF image [R/128, 128(c_in), R... ]

    img[rt, ci, t*128 + r] = W[rt*128 + r, t*128 + ci]  as bf16 (+-1).
    For row-tile rt, the [128, C] slice DMAs contiguously into SBUF and
    column block t is the stationary [K=128, M=128] operand of matmul.
    """
    w = (2 * w_bits - 1).astype(np.float32)
    R, C = w.shape
    img = w.reshape(R // P, P, C // P, P).transpose(0, 3, 2, 1)  # [rt, ci, t, r]
    return np.ascontiguousarray(img.reshape(R // P, P, C)).astype(ml_dtypes.bfloat16)


def _scale_img(v: np.ndarray) -> np.ndarray:
    """[4096] -> [128, 32] with img[p, t] = v[t*128 + p]."""
    return np.ascontiguousarray(v.reshape(-1, P).T.astype(np.float32))


def _build():
    """Build + compile the per-core Bass kernel (shared by all 8 cores)."""
    import concourse.bacc as bacc
    import concourse.tile as tile
    import concourse.mybir as mybir

    dt = mybir.dt
    nc = bacc.Bacc("TRN2", target_bir_lowering=False, debug=False,
                   enable_asserts=False, num_devices=NCORES,
                   enable_partition_id=False)

    xt_d = nc.dram_tensor("xt", [IN, NPC], dt.bfloat16, kind="ExternalInput").ap()
    w1_d = nc.dram_tensor("w1p", [MT, P, IN], dt.bfloat16, kind="ExternalInput").ap()
    w3_d = nc.dram_tensor("w3p", [OT, P, MID], dt.bfloat16, kind="ExternalInput").ap()
    s0_d = nc.dram_tensor("s0i", [P, KT], dt.float32, kind="ExternalInput").ap()
    s2_d = nc.dram_tensor("s2i", [P, MT], dt.float32, kind="ExternalInput").ap()
    s4_d = nc.dram_tensor("s4i", [P, OT], dt.float32, kind="ExternalInput").ap()
    bi_d = nc.dram_tensor("bi", [P, OT], dt.float32, kind="ExternalInput").ap()
    out_d = nc.dram_tensor("outt", [OUT, NPC], dt.float32, kind="ExternalOutput").ap()

    G = 4  # mt-tiles in the t-major opening wave (4 x [128,1024] = 8 PSUM banks)

    with tile.TileContext(nc) as tc:
        with (
            tc.tile_pool(name="const", bufs=1) as const,
            tc.tile_pool(name="xs_pool", bufs=KT) as xs_pool,
            tc.tile_pool(name="h_pool", bufs=MT) as h_pool,
            tc.tile_pool(name="w_pool", bufs=6) as w_pool,
            tc.tile_pool(name="xin_pool", bufs=3) as xin_pool,
            tc.tile_pool(name="out_pool", bufs=2) as out_pool,
            tc.tile_pool(name="ps_pool", bufs=G, space="PSUM") as ps_pool,
        ):
            s0t = const.tile([P, KT], dt.float32, name="s0t")
            s2t = const.tile([P, MT], dt.float32, name="s2t")
            s4t = const.tile([P, OT], dt.float32, name="s4t")
            bt = const.tile([P, OT], dt.float32, name="bt")

            # Stage 1: stream x shard (feature-major bf16), scale by s0.
            # DMA issue order is the critical path to the first matmul:
            # wave-weight chunk 0 (t=0..7 slices) for all G images, then s0
            # and x tile 0, then the rest interleaved. s2/s4/bias are not
            # needed until the first PSUM drain — deferred.
            CH = 4
            CW = IN // CH  # weight-image chunk: 8 t-slices, 256 KiB
            wave_w = [w_pool.tile([P, IN], dt.bfloat16, name=f"w1t{g}", tag="w")
                      for g in range(G)]
            for g in range(G):
                nc.sync.dma_start(wave_w[g][:, 0:CW], w1_d[g, :, 0:CW])
            nc.sync.dma_start(s0t[:], s0_d[:])

            xs_tiles = []
            for t in range(KT):
                xf = xin_pool.tile([P, NPC], dt.bfloat16, name=f"xf{t}", tag="xf")
                nc.sync.dma_start(xf[:], xt_d[t * P:(t + 1) * P, :])
                xs = xs_pool.tile([P, NPC], dt.bfloat16, name=f"xs{t}", tag="xs")
                nc.vector.tensor_scalar_mul(xs[:], xf[:], s0t[:, t:t + 1])
                xs_tiles.append(xs)
                if t in (2, 4, 6):  # wave-weight chunks 1..3
                    c = t // 2
                    for g in range(G):
                        nc.sync.dma_start(wave_w[g][:, c * CW:(c + 1) * CW],
                                          w1_d[g, :, c * CW:(c + 1) * CW])
                if t == 8:
                    nc.sync.dma_start(s2t[:], s2_d[:])
                    nc.sync.dma_start(s4t[:], s4_d[:])
                    nc.sync.dma_start(bt[:], bi_d[:])

            # Stage 2: h.T = W1 @ xs (per 128-row tile of MID), * s2, -> bf16.
            # Opening wave: mt = 0..G-1 t-major, consuming x as it arrives.
            h_tiles = []
            wave_ps = [ps_pool.tile([P, NPC], dt.float32, name=f"ps1{g}", tag="ps")
                       for g in range(G)]
            for t in range(KT):
                for g in range(G):
                    lhsT = wave_w[g][:, t * P:(t + 1) * P]
                    for f in range(NPC // FD):
                        nc.tensor.matmul(
                            wave_ps[g][:, f * FD:(f + 1) * FD], lhsT,
                            xs_tiles[t][:, f * FD:(f + 1) * FD],
                            start=(t == 0), stop=(t == KT - 1),
                        )
            for g in range(G):
                h2 = h_pool.tile([P, NPC], dt.bfloat16, name=f"h{g}", tag="h")
                nc.vector.tensor_scalar_mul(h2[:], wave_ps[g][:], s2t[:, g:g + 1])
                h_tiles.append(h2)

            # Remaining mt tiles: mt-major (all xs resident by now).
            for mt in range(G, MT):
                wt = w_pool.tile([P, IN], dt.bfloat16, name=f"w1t{mt}", tag="w")
                nc.sync.dma_start(wt[:], w1_d[mt, :, :])
                ps = ps_pool.tile([P, NPC], dt.float32, name=f"ps1{mt}", tag="ps")
                for t in range(KT):
                    lhsT = wt[:, t * P:(t + 1) * P]
                    for f in range(NPC // FD):
                        nc.tensor.matmul(
                            ps[:, f * FD:(f + 1) * FD], lhsT,
                            xs_tiles[t][:, f * FD:(f + 1) * FD],
                            start=(t == 0), stop=(t == KT - 1),
                        )
                h2 = h_pool.tile([P, NPC], dt.bfloat16, name=f"h{mt}", tag="h")
                nc.vector.tensor_scalar_mul(h2[:], ps[:], s2t[:, mt:mt + 1])
                h_tiles.append(h2)

            # Stage 3: out.T = W3 @ h, * s4 + bias, DMA out.
            for ot in range(OT):
                wt = w_pool.tile([P, MID], dt.bfloat16, name=f"w3t{ot}", tag="w")
                nc.sync.dma_start(wt[:], w3_d[ot, :, :])
                ps = ps_pool.tile([P, NPC], dt.float32, name=f"ps2{ot}", tag="ps")
                for t in range(MT):
                    lhsT = wt[:, t * P:(t + 1) * P]
                    for f in range(NPC // FD):
                        nc.tensor.matmul(
                            ps[:, f * FD:(f + 1) * FD], lhsT,
                            h_tiles[t][:, f * FD:(f + 1) * FD],
                            start=(t == 0), stop=(t == MT - 1),
                        )
                ob = out_pool.tile([P, NPC], dt.float32, name=f"ob{ot}", tag="ob")
                nc.vector.tensor_scalar(
                    ob[:], ps[:], s4t[:, ot:ot + 1], bt[:, ot:ot + 1],
                    mybir.AluOpType.mult, mybir.AluOpType.add,
                )
                nc.sync.dma_start(out_d[ot * P:(ot + 1) * P, :], ob[:])

    nc.compile()
    return nc


def run(inputs: dict, trace: bool = False):
    """Run on 8 cores; returns (out [B,S,OUT] fp32, BassKernelResults)."""
    from concourse.bass_utils import run_bass_kernel_spmd

    if "nc" not in _cache:
        _cache["nc"] = _build()
    nc = _cache["nc"]

    x = np.asarray(inputs["x"], dtype=np.float32)
    w1p = _pack_weight(np.asarray(inputs["w1_bits"]))
    w3p = _pack_weight(np.asarray(inputs["w3_bits"]))
    s0i = _scale_img(np.asarray(inputs["scaling0"]))
    s2i = _scale_img(np.asarray(inputs["scaling2"]))
    s4i = _scale_img(np.asarray(inputs["scaling4"]))
    bi = _scale_img(np.asarray(inputs["bias"]))

    xT = np.ascontiguousarray(x.reshape(NTOK, IN).T).astype(ml_dtypes.bfloat16)
    in_maps = []
    for c in range(NCORES):
        in_maps.append({
            "xt": np.ascontiguousarray(xT[:, c * NPC:(c + 1) * NPC]),
            "w1p": w1p, "w3p": w3p,
            "s0i": s0i, "s2i": s2i, "s4i": s4i, "bi": bi,
        })

    res = run_bass_kernel_spmd(nc, in_maps, core_ids=list(range(NCORES)),
                               trace=trace)
    outT = np.concatenate([res.results[c]["outt"] for c in range(NCORES)],
                          axis=1)  # [OUT, NTOK]
    out = np.ascontiguousarray(outT.T).reshape(B, S, OUT)
    return out, res


def kernel(**inputs) -> np.ndarray:
    out, _ = run(inputs)
    return out


# revision 9
# speedup vs baseline: 1.0088x; 1.0088x over previous
"""DBF (binary-weight) MLP kernel for 8 TRN2 NeuronCores.

Computation (see reference):
    h   = (x * s0) @ W1.T          W1 = 2*w1_bits - 1  (+-1)
    h   = h * s2
    out = h @ W3.T * s4 + bias     W3 = 2*w3_bits - 1  (+-1)

Strategy:
  - Data-parallel: shard the 4*2048 = 8192 tokens across 8 cores (1024 each).
    Weights/scalings replicated. No collectives.
  - Activations kept feature-major on chip ([feature, token]); both GEMMs then
    chain naturally on the tensor engine (contraction dim on partitions) with
    no transposes between layers.
  - +-1 weights are exact in bf16, so both GEMMs run in bf16 (moving operand
    = activations rounded to bf16; fp32 PSUM accumulation).
  - Weights are packed on the host into per-m-tile SBUF images so every DMA
    is a single fully contiguous 1 MiB transfer.
"""

import numpy as np
import ml_dtypes

B, S, IN, MID, OUT = 4, 2048, 4096, 4096, 4096
NCORES = 8
NTOK = B * S            # 8192 tokens
NPC = NTOK // NCORES    # 1024 tokens per core
P = 128
KT, MT, OT = IN // P, MID // P, OUT // P   # 32 tiles each
FD = 512                # matmul moving free dim (one PSUM bank of fp32)

_cache = {}


def _pack_weight(w_bits: np.ndarray) -> np.ndarray:
    """[R, C] 0/1 int32 -> per-row-tile SBUF image [R/128, 128(c_in), R... ]

    img[rt, ci, t*128 + r] = W[rt*128 + r, t*128 + ci]  as bf16 (+-1).
    For row-tile rt, the [128, C] slice DMAs contiguously into SBUF and
    column block t is the stationary [K=128, M=128] operand of matmul.
    """
    w = (2 * w_bits - 1).astype(np.float32)
    R, C = w.shape
    img = w.reshape(R // P, P, C // P, P).transpose(0, 3, 2, 1)  # [rt, ci, t, r]
    return np.ascontiguousarray(img.reshape(R // P, P, C)).astype(ml_dtypes.bfloat16)


def _scale_img(v: np.ndarray) -> np.ndarray:
    """[4096] -> [128, 32] with img[p, t] = v[t*128 + p]."""
    return np.ascontiguousarray(v.reshape(-1, P).T.astype(np.float32))


def _build():
    """Build + compile the per-core Bass kernel (shared by all 8 cores)."""
    import concourse.bacc as bacc
    import concourse.tile as tile
    import concourse.mybir as mybir

    dt = mybir.dt
    nc = bacc.Bacc("TRN2", target_bir_lowering=False, debug=False,
                   enable_asserts=False, num_devices=NCORES,
                   enable_partition_id=False)

    xt_d = nc.dram_tensor("xt", [IN, NPC], dt.bfloat16, kind="ExternalInput").ap()
    w1_d = nc.dram_tensor("w1p", [MT, P, IN], dt.bfloat16, kind="ExternalInput").ap()
    w3_d = nc.dram_tensor("w3p", [OT, P, MID], dt.bfloat16, kind="ExternalInput").ap()
    s0_d = nc.dram_tensor("s0i", [P, KT], dt.float32, kind="ExternalInput").ap()
    s2_d = nc.dram_tensor("s2i", [P, MT], dt.float32, kind="ExternalInput").ap()
    s4_d = nc.dram_tensor("s4i", [P, OT], dt.float32, kind="ExternalInput").ap()
    bi_d = nc.dram_tensor("bi", [P, OT], dt.float32, kind="ExternalInput").ap()
    out_d = nc.dram_tensor("outt", [OUT, NPC], dt.float32, kind="ExternalOutput").ap()

    G = 4  # mt-tiles in the t-major opening wave (4 x [128,1024] = 8 PSUM banks)

    with tile.TileContext(nc) as tc:
        with (
            tc.tile_pool(name="const", bufs=1) as const,
            tc.tile_pool(name="xs_pool", bufs=KT) as xs_pool,
            tc.tile_pool(name="h_pool", bufs=MT) as h_pool,
            tc.tile_pool(name="w_pool", bufs=6) as w_pool,
            tc.tile_pool(name="xin_pool", bufs=3) as xin_pool,
            tc.tile_pool(name="out_pool", bufs=2) as out_pool,
            tc.tile_pool(name="ps_pool", bufs=G, space="PSUM") as ps_pool,
        ):
            s0t = const.tile([P, KT], dt.float32, name="s0t")
            s2t = const.tile([P, MT], dt.float32, name="s2t")
            s4t = const.tile([P, OT], dt.float32, name="s4t")
            bt = const.tile([P, OT], dt.float32, name="bt")

            # Stage 1: stream x shard (feature-major bf16), scale by s0.
            # DMA issue order is the critical path to the first matmul:
            # wave-weight chunk 0 (t=0..7 slices) for all G images, then s0
            # and x tile 0, then the rest interleaved. s2/s4/bias are not
            # needed until the first PSUM drain — deferred.
            # Weights ride the Activation HWDGE queue (nc.scalar), x/out the
            # SP queue (nc.sync) — two parallel DMA streams. Wave weight
            # images are chunked so the first matmul waits on 256 KiB only.
            CH = 4
            CW = IN // CH  # weight-image chunk: 8 t-slices, 256 KiB
            wave_w = [w_pool.tile([P, IN], dt.bfloat16, name=f"w1t{g}", tag="w")
                      for g in range(G)]
            for c in range(CH):
                for g in range(G):
                    nc.scalar.dma_start(wave_w[g][:, c * CW:(c + 1) * CW],
                                        w1_d[g, :, c * CW:(c + 1) * CW])
            nc.sync.dma_start(s0t[:], s0_d[:])

            xs_tiles = []
            for t in range(KT):
                xf = xin_pool.tile([P, NPC], dt.bfloat16, name=f"xf{t}", tag="xf")
                nc.sync.dma_start(xf[:], xt_d[t * P:(t + 1) * P, :])
                xs = xs_pool.tile([P, NPC], dt.bfloat16, name=f"xs{t}", tag="xs")
                nc.vector.tensor_scalar_mul(xs[:], xf[:], s0t[:, t:t + 1])
                xs_tiles.append(xs)
                if t == 8:
                    nc.sync.dma_start(s2t[:], s2_d[:])
                    nc.sync.dma_start(s4t[:], s4_d[:])
                    nc.sync.dma_start(bt[:], bi_d[:])

            # Stage 2: h.T = W1 @ xs (per 128-row tile of MID), * s2, -> bf16.
            # Opening wave: mt = 0..G-1 t-major, consuming x as it arrives.
            h_tiles = []
            wave_ps = [ps_pool.tile([P, NPC], dt.float32, name=f"ps1{g}", tag="ps")
                       for g in range(G)]
            for t in range(KT):
                for g in range(G):
                    lhsT = wave_w[g][:, t * P:(t + 1) * P]
                    for f in range(NPC // FD):
                        nc.tensor.matmul(
                            wave_ps[g][:, f * FD:(f + 1) * FD], lhsT,
                            xs_tiles[t][:, f * FD:(f + 1) * FD],
                            start=(t == 0), stop=(t == KT - 1),
                        )
            for g in range(G):
                h2 = h_pool.tile([P, NPC], dt.bfloat16, name=f"h{g}", tag="h")
                nc.vector.tensor_scalar_mul(h2[:], wave_ps[g][:], s2t[:, g:g + 1])
                h_tiles.append(h2)

            # Remaining mt tiles: mt-major (all xs resident by now).
            for mt in range(G, MT):
                wt = w_pool.tile([P, IN], dt.bfloat16, name=f"w1t{mt}", tag="w")
                nc.scalar.dma_start(wt[:], w1_d[mt, :, :])
                ps = ps_pool.tile([P, NPC], dt.float32, name=f"ps1{mt}", tag="ps")
                for t in range(KT):
                    lhsT = wt[:, t * P:(t + 1) * P]
                    for f in range(NPC // FD):
                        nc.tensor.matmul(
                            ps[:, f * FD:(f + 1) * FD], lhsT,
                            xs_tiles[t][:, f * FD:(f + 1) * FD],
                            start=(t == 0), stop=(t == KT - 1),
                        )
                h2 = h_pool.tile([P, NPC], dt.bfloat16, name=f"h{mt}", tag="h")
                nc.vector.tensor_scalar_mul(h2[:], ps[:], s2t[:, mt:mt + 1])
                h_tiles.append(h2)

            # Stage 3: out.T = W3 @ h, * s4 + bias, DMA out.
            for ot in range(OT):
                wt = w_pool.tile([P, MID], dt.bfloat16, name=f"w3t{ot}", tag="w")
                nc.scalar.dma_start(wt[:], w3_d[ot, :, :])
                ps = ps_pool.tile([P, NPC], dt.float32, name=f"ps2{ot}", tag="ps")
                for t in range(MT):
                    lhsT = wt[:, t * P:(t + 1) * P]
                    for f in range(NPC // FD):
                        nc.tensor.matmul(
                            ps[:, f * FD:(f + 1) * FD], lhsT,
                            h_tiles[t][:, f * FD:(f + 1) * FD],
                            start=(t == 0), stop=(t == MT - 1),
                        )
                ob = out_pool.tile([P, NPC], dt.float32, name=f"ob{ot}", tag="ob")
                nc.vector.tensor_scalar(
                    ob[:], ps[:], s4t[:, ot:ot + 1], bt[:, ot:ot + 1],
                    mybir.AluOpType.mult, mybir.AluOpType.add,
                )
                nc.sync.dma_start(out_d[ot * P:(ot + 1) * P, :], ob[:])

    nc.compile()
    return nc


def run(inputs: dict, trace: bool = False):
    """Run on 8 cores; returns (out [B,S,OUT] fp32, BassKernelResults)."""
    from concourse.bass_utils import run_bass_kernel_spmd

    if "nc" not in _cache:
        _cache["nc"] = _build()
    nc = _cache["nc"]

    x = np.asarray(inputs["x"], dtype=np.float32)
    w1p = _pack_weight(np.asarray(inputs["w1_bits"]))
    w3p = _pack_weight(np.asarray(inputs["w3_bits"]))
    s0i = _scale_img(np.asarray(inputs["scaling0"]))
    s2i = _scale_img(np.asarray(inputs["scaling2"]))
    s4i = _scale_img(np.asarray(inputs["scaling4"]))
    bi = _scale_img(np.asarray(inputs["bias"]))

    xT = np.ascontiguousarray(x.reshape(NTOK, IN).T).astype(ml_dtypes.bfloat16)
    in_maps = []
    for c in range(NCORES):
        in_maps.append({
            "xt": np.ascontiguousarray(xT[:, c * NPC:(c + 1) * NPC]),
            "w1p": w1p, "w3p": w3p,
            "s0i": s0i, "s2i": s2i, "s4i": s4i, "bi": bi,
        })

    res = run_bass_kernel_spmd(nc, in_maps, core_ids=list(range(NCORES)),
                               trace=trace)
    outT = np.concatenate([res.results[c]["outt"] for c in range(NCORES)],
                          axis=1)  # [OUT, NTOK]
    out = np.ascontiguousarray(outT.T).reshape(B, S, OUT)
    return out, res


def kernel(**inputs) -> np.ndarray:
    out, _ = run(inputs)
    return out
